# revision 1
# baseline (speedup 1.0000x reference)
"""Trainium2 Bass kernel for nn_AttentionBlock (adaLN-modulated GroupNorm attention).

Sharding: data-parallel over batch B=8 -> one batch per NeuronCore (8 cores).
Each core runs the full block for its batch:
  groupnorm(32 groups) -> adaLN modulate -> qkv matmul -> 8-head attention
  (softmax over keys) -> proj matmul -> gated residual.

Layouts (per core):
  x (fp32), xm (bf16):  [C=512, T=1024] as 4 tiles [128, 1024], channels on
                        partitions.
  qkv out (bf16): 12 tiles [128, 1024], channel order PERMUTED to type-major
              [q0..q7 | k0..k7 | v0..v7] (64 rows per head-type block) so that
              q_h and k_h always live at the same partition offset (0 or 64).
  scoresT:    [s, t] per head ([128 s, 1024 t] PSUM fp32), one batched exp on
              ScalarE fuses the PSUM->SBUF eviction (out bf16).
  PV:         U[65, t] = [vT | ones].T @ expT  -> row 64 is the softmax denom.
  normalize:  1/denom via reciprocal_approx_fast on partition 0, broadcast
              across partitions with gpsimd.partition_broadcast.

Matmuls run in bf16 (fp32 PSUM accumulation); groupnorm statistics stay fp32.
"""

import numpy as np

import concourse.bass as bass
import concourse.tile as tile
from concourse import bacc, mybir
from concourse.bass_utils import run_bass_kernel_spmd

AF = mybir.ActivationFunctionType
f32 = mybir.dt.float32
bf16 = mybir.dt.bfloat16

B, C, HH, WW, E = 8, 512, 32, 32, 512
HEADS, G = 8, 32
T = HH * WW          # 1024
CH = C // HEADS      # 64
NC_ = C // 128       # 4 channel chunks
NO = 3 * C // 128    # 12 qkv output chunks
NT = T // 512        # 2 t-chunks of 512
NS = T // 128        # 8 s-chunks of 128
EPS = 1e-5


def _perm():
    """new[512*ty + 64*h + r] = orig[192*h + 64*ty + r] (head-major -> type-major)."""
    p = np.empty(3 * C, np.int64)
    for h in range(HEADS):
        for ty in range(3):
            p[512 * ty + 64 * h : 512 * ty + 64 * h + 64] = (
                192 * h + 64 * ty + np.arange(64)
            )
    return p


def _build_program():
    nc = bacc.Bacc("TRN2", target_bir_lowering=False, debug=False, num_devices=8)

    # ---- DRAM parameters (per-core shards; weights replicated, bf16) ----
    x_d = nc.declare_dram_parameter("x", [C, T], f32, isOutput=False)
    emb_d = nc.declare_dram_parameter("emb", [E], f32, isOutput=False)
    qw_d = nc.declare_dram_parameter("qkv_wT", [C, 3 * C], bf16, isOutput=False)
    qb_d = nc.declare_dram_parameter("qkv_b", [3 * C], f32, isOutput=False)
    aw_d = nc.declare_dram_parameter("ada_wT", [E, 3 * C], bf16, isOutput=False)
    ab_d = nc.declare_dram_parameter("ada_b", [3 * C], f32, isOutput=False)
    pw_d = nc.declare_dram_parameter("proj_wT", [C, C], bf16, isOutput=False)
    pb_d = nc.declare_dram_parameter("proj_b", [C], f32, isOutput=False)
    gind_d = nc.declare_dram_parameter("gind", [128, 8], f32, isOutput=False)
    gindT_d = nc.declare_dram_parameter("gindT", [8, 128], f32, isOutput=False)
    ident_d = nc.declare_dram_parameter("ident", [128, 128], bf16, isOutput=False)
    ones_d = nc.declare_dram_parameter("ones", [128, 64], f32, isOutput=False)
    out_d = nc.declare_dram_parameter("out", [C, T], f32, isOutput=True)

    from contextlib import ExitStack

    with tile.TileContext(nc) as tc, ExitStack() as ctx:
        ctx.enter_context(
            nc.allow_low_precision(reason="bf16 matmul inputs; fp32 accumulate")
        )
        P = ctx.enter_context(tc.tile_pool(name="persist", bufs=1))
        # one shared PSUM tag: 2 rotating [128,1024] fp32 slots (4 banks)
        PSM = ctx.enter_context(tc.tile_pool(name="psm", bufs=2, space="PSUM"))
        PSU = ctx.enter_context(tc.tile_pool(name="psu", bufs=4, space="PSUM"))

        # ---- persistent SBUF tiles + input DMAs ----
        gind_sb = P.tile([128, 8], f32, tag="gind")
        gindT_sb = P.tile([8, 128], f32, tag="gindT")
        ident_sb = P.tile([128, 128], bf16, tag="ident")
        ones_sb = P.tile([128, 64], f32, tag="ones")
        emb_sb = P.tile([128, 4], f32, tag="emb")
        silu_sb = P.tile([128, 4], bf16, tag="silu")
        qb_sb = P.tile([128, 12], f32, tag="qb")
        ab_sb = P.tile([128, 12], f32, tag="ab")
        pb_sb = P.tile([128, 4], f32, tag="pb")
        mod_sb = P.tile([128, 12], f32, tag="mod")

        nc.sync.dma_start(out=gind_sb, in_=gind_d.ap())
        nc.sync.dma_start(out=gindT_sb, in_=gindT_d.ap())
        nc.sync.dma_start(out=ident_sb, in_=ident_d.ap())
        nc.sync.dma_start(out=ones_sb, in_=ones_d.ap())
        nc.sync.dma_start(out=emb_sb, in_=emb_d.ap().rearrange("(f p) -> p f", p=128))
        nc.sync.dma_start(out=qb_sb, in_=qb_d.ap().rearrange("(f p) -> p f", p=128))
        nc.sync.dma_start(out=ab_sb, in_=ab_d.ap().rearrange("(f p) -> p f", p=128))
        nc.sync.dma_start(out=pb_sb, in_=pb_d.ap().rearrange("(f p) -> p f", p=128))

        xf = []
        for i in range(NC_):
            t_ = P.tile([128, T], f32, tag=f"xf{i}")
            nc.sync.dma_start(out=t_, in_=x_d.ap()[128 * i : 128 * (i + 1), :])
            xf.append(t_)
        awp_cm = tc.tile_pool(name="awp", bufs=1)
        AWP = awp_cm.__enter__()
        aw = []
        for j in range(4):
            t_ = AWP.tile([128, 3 * C], bf16, tag=f"aw{j}", name=f"aw{j}")
            nc.sync.dma_start(out=t_, in_=aw_d.ap()[128 * j : 128 * (j + 1), :])
            aw.append(t_)
        qw = []
        for j in range(4):
            t_ = P.tile([128, 3 * C], bf16, tag=f"qw{j}")
            nc.sync.dma_start(out=t_, in_=qw_d.ap()[128 * j : 128 * (j + 1), :])
            qw.append(t_)
        pw = []
        for j in range(4):
            t_ = P.tile([128, C], bf16, tag=f"pw{j}")
            nc.sync.dma_start(out=t_, in_=pw_d.ap()[128 * j : 128 * (j + 1), :])
            pw.append(t_)

        # ---- phase 1: adaLN modulation (PE) + groupnorm stats (DVE) ----
        sg_sb = P.tile([128, 4], f32, tag="sg")
        nc.scalar.activation(sg_sb, emb_sb, AF.Sigmoid)
        nc.vector.tensor_mul(silu_sb, emb_sb, sg_sb)
        # mod^T = silu^T @ ada_wT as [1, 1536], then DRAM-bounce to [128, 12]
        mrow = P.tile([1, 3 * C], f32, tag="mrow")
        for oc in range(3):
            mps = PSM.tile([1, 512], f32, tag="sc", name=f"mps{oc}")
            for j in range(4):
                nc.tensor.matmul(
                    mps,
                    silu_sb[:, j : j + 1],
                    aw[j][:, 512 * oc : 512 * (oc + 1)],
                    start=(j == 0),
                    stop=(j == 3),
                )
            nc.vector.tensor_copy(mrow[:, 512 * oc : 512 * (oc + 1)], mps)
        awp_cm.__exit__(None, None, None)
        EXPP = ctx.enter_context(tc.tile_pool(name="expp", bufs=6))
        ANP = ctx.enter_context(tc.tile_pool(name="anp", bufs=4))
        modp_sb = P.tile([128, 12], f32, tag="modp")
        # partition-scatter via DRAM bounce (SBUF partition dim is physical)
        DP = ctx.enter_context(tc.tile_pool(name="dramp", bufs=1, space="DRAM"))
        mod_scr = DP.tile([1, 3 * C], f32, tag="modscr")
        nc.sync.dma_start(out=mod_scr, in_=mrow)
        nc.sync.dma_start(
            out=modp_sb, in_=mod_scr[0, :].rearrange("(f p) -> p f", p=128)
        )
        nc.vector.tensor_add(mod_sb, modp_sb, ab_sb)

        mv = []
        for i in range(NC_):
            st6 = P.tile([128, 2, 6], f32, tag=f"st6{i}")
            xv = xf[i][:].rearrange("p (s f) -> p s f", f=512)
            for si in range(2):
                nc.vector.bn_stats(st6[:, si, :], xv[:, si, :])
            mv_i = P.tile([128, 2], f32, tag=f"mv{i}")
            nc.vector.bn_aggr(mv_i, st6)
            # E2 = var + mu^2 into col 1
            tm = P.tile([128, 1], f32, tag=f"tmu{i}")
            nc.vector.tensor_mul(tm, mv_i[:, 0:1], mv_i[:, 0:1])
            nc.vector.tensor_add(mv_i[:, 1:2], mv_i[:, 1:2], tm)
            mv.append(mv_i)

        stats8_ps = PSM.tile([8, 8], f32, tag="sc", name="stats8")
        for i in range(NC_):
            nc.tensor.matmul(
                stats8_ps[:, 2 * i : 2 * i + 2], gind_sb, mv[i], start=True, stop=True
            )
        s8 = P.tile([8, 8], f32, tag="s8")
        nc.vector.tensor_copy(s8, stats8_ps)
        musq8 = P.tile([8, 4], f32, tag="musq8")
        var8 = P.tile([8, 4], f32, tag="var8")
        sd8 = P.tile([8, 4], f32, tag="sd8")
        rstd8 = P.tile([8, 4], f32, tag="rstd8")
        for i in range(NC_):
            nc.vector.tensor_mul(
                musq8[:, i : i + 1], s8[:, 2 * i : 2 * i + 1], s8[:, 2 * i : 2 * i + 1]
            )
            nc.vector.tensor_sub(
                var8[:, i : i + 1], s8[:, 2 * i + 1 : 2 * i + 2], musq8[:, i : i + 1]
            )
        eps8 = P.tile([8, 1], f32, tag="eps8")
        nc.vector.memset(eps8, EPS)
        nc.scalar.activation(sd8, var8, AF.Sqrt, bias=eps8)
        nc.vector.reciprocal(rstd8, sd8)

        xm = []
        for i in range(NC_):
            statbc = PSM.tile([128, 2], f32, tag="sc", name=f"statbc{i}")
            nc.tensor.matmul(
                statbc[:, 0:1], gindT_sb, s8[:, 2 * i : 2 * i + 1], start=True, stop=True
            )
            nc.tensor.matmul(
                statbc[:, 1:2], gindT_sb, rstd8[:, i : i + 1], start=True, stop=True
            )
            s1p = P.tile([128, 1], f32, tag=f"s1p{i}")
            A_i = P.tile([128, 1], f32, tag=f"A{i}")
            B_i = P.tile([128, 1], f32, tag=f"B{i}")
            tm2 = P.tile([128, 1], f32, tag=f"tm2{i}")
            nc.vector.tensor_scalar_add(s1p, mod_sb[:, 4 + i : 5 + i], 1.0)
            nc.vector.tensor_mul(A_i, statbc[:, 1:2], s1p)
            nc.vector.tensor_mul(tm2, statbc[:, 0:1], A_i)
            nc.vector.tensor_sub(B_i, mod_sb[:, i : i + 1], tm2)
            xm_i = P.tile([128, T], bf16, tag=f"xm{i}")
            nc.scalar.activation(xm_i, xf[i], AF.Identity, bias=B_i, scale=A_i)
            xm.append(xm_i)

        # ---- phase 2: qkv matmul [1536, 1024] (channel order = type-major) ----
        qkv = [P.tile([128, T], bf16, tag=f"qkv{m}", name=f"qkv{m}") for m in range(NO)]
        # chunk order: all three chunks of head pair 0 first, then pair 1, ...
        m_order = [p + 4 * ty for p in range(4) for ty in range(3)]
        for m in m_order:
            ps = PSM.tile([128, T], f32, tag="sc", name=f"qkvps{m}")
            for t in range(NT):
                for j in range(4):
                    nc.tensor.matmul(
                        ps[:, 512 * t : 512 * (t + 1)],
                        qw[j][:, 128 * m : 128 * (m + 1)],
                        xm[j][:, 512 * t : 512 * (t + 1)],
                        start=(j == 0),
                        stop=(j == 3),
                    )
            nc.vector.tensor_scalar_add(qkv[m][:], ps, qb_sb[:, m : m + 1])

        # ---- phase 3+4: attention, head pairs interleaved ----
        # Heads 2j / 2j+1 live at partition offsets 0 / 64 of the same qkv
        # tiles; interleaving their K=64 matmuls puts them in different PE
        # row-groups so they can execute concurrently.
        a_sb = [
            P.tile([128, T], bf16, tag=f"asb{j}", name=f"asb{j}") for j in range(NC_)
        ]
        vT = [
            P.tile([128, 8, 65], bf16, tag=f"vt{h}", name=f"vt{h}")
            for h in range(HEADS)
        ]
        for hp in range(4):
            heads = (2 * hp, 2 * hp + 1)
            for h in heads:
                nc.vector.tensor_copy(
                    vT[h][:, :, 64:65],
                    ones_sb[:, 0:8].rearrange("p (a o) -> p a o", o=1),
                )
            for s in range(NS):
                for h in heads:
                    off = 64 * (h % 2)
                    v_ap = qkv[8 + h // 2][off : off + 64, :]
                    vtr = PSM.tile([128, 64], bf16, tag="sc", name=f"vtr{hp}_{s}_{h}")
                    nc.tensor.transpose(
                        vtr,
                        v_ap[:, 128 * s : 128 * (s + 1)],
                        ident_sb[off : off + 64, off : off + 64],
                        tile_position=(off, 0),
                    )
                    nc.vector.tensor_copy(vT[h][:, s, 0:64], vtr)
            U = {}
            for h in heads:
                for t in range(NT):
                    U[(h, t)] = PSU.tile([65, 512], f32, tag="u", name=f"u{h}_{t}")
            ex_tiles = {}
            for s in range(NS):
                for h in heads:
                    off = 64 * (h % 2)
                    q_ap = qkv[h // 2][off : off + 64, :]
                    k_ap = qkv[4 + h // 2][off : off + 64, :]
                    sc = PSM.tile([128, T], f32, tag="sc", name=f"sc{hp}_{s}_{h}")
                    for t in range(NT):
                        nc.tensor.matmul(
                            sc[:, 512 * t : 512 * (t + 1)],
                            k_ap[:, 128 * s : 128 * (s + 1)],
                            q_ap[:, 512 * t : 512 * (t + 1)],
                            start=True,
                            stop=True,
                            tile_position=(off, 0),
                        )
                    ex = EXPP.tile([128, T], bf16, tag="ex")
                    nc.scalar.activation(ex, sc, AF.Exp, scale=0.125)
                    ex_tiles[(h, s)] = ex
                if s >= 1:
                    for h in heads:
                        ex = ex_tiles.pop((h, s - 1))
                        for t in range(NT):
                            nc.tensor.matmul(
                                U[(h, t)],
                                vT[h][:, s - 1, :],
                                ex[:, 512 * t : 512 * (t + 1)],
                                start=(s - 1 == 0),
                                stop=False,
                            )
            for h in heads:
                ex = ex_tiles.pop((h, NS - 1))
                for t in range(NT):
                    nc.tensor.matmul(
                        U[(h, t)],
                        vT[h][:, NS - 1, :],
                        ex[:, 512 * t : 512 * (t + 1)],
                        start=False,
                        stop=True,
                    )
            # normalize: a = U[0:64] / denom (denom = row 64); the reciprocal
            # runs on partition 0 (partition_broadcast sources partition 0)
            for h in heads:
                off = 64 * (h % 2)
                for t in range(NT):
                    rc = ANP.tile([65, 512], f32, tag="rc", bufs=2)
                    nc.vector.tensor_copy(rc[64:65, :], U[(h, t)][64:65, :])
                    rc0 = ANP.tile([1, 512], f32, tag="rc0", bufs=2)
                    nc.sync.dma_start(out=rc0, in_=rc[64:65, :])
                    nc.vector.reciprocal_approx_fast(out=rc0[:], in_=rc0[:])
                    rbs = ANP.tile([64, 512], f32, tag="rbs")
                    nc.gpsimd.partition_broadcast(rbs[:], rc0[:])
                    abf = ANP.tile([64, 512], bf16, tag="abf")
                    nc.vector.tensor_mul(abf, U[(h, t)][0:64, :], rbs)
                    nc.sync.dma_start(
                        out=a_sb[h // 2][off : off + 64, 512 * t : 512 * (t + 1)],
                        in_=abf,
                    )

        # ---- phase 5: proj + gated residual ----
        pbg = []
        for i_ in range(NC_):
            t_ = P.tile([128, 1], f32, tag=f"pbg{i_}")
            nc.vector.tensor_mul(t_, pb_sb[:, i_ : i_ + 1], mod_sb[:, 8 + i_ : 9 + i_])
            pbg.append(t_)
        for m in range(NC_):
            ps = PSM.tile([128, T], f32, tag="sc", name=f"projps{m}")
            for t in range(NT):
                for j in range(4):
                    nc.tensor.matmul(
                        ps[:, 512 * t : 512 * (t + 1)],
                        pw[j][:, 128 * m : 128 * (m + 1)],
                        a_sb[j][:, 512 * t : 512 * (t + 1)],
                        start=(j == 0),
                        stop=(j == 3),
                    )
            tg = ANP.tile([128, T], f32, tag="tg", bufs=2)
            nc.scalar.activation(
                tg, ps, AF.Identity, bias=pbg[m], scale=mod_sb[:, 8 + m : 9 + m]
            )
            # residual in-place into xf (xf never feeds a matmul)
            nc.vector.tensor_add(xf[m][:], xf[m][:], tg)
            nc.sync.dma_start(out=out_d.ap()[128 * m : 128 * (m + 1), :], in_=xf[m])

    nc.compile()
    return nc


_PROGRAM = None
LAST_RESULTS = None


def _get_program():
    global _PROGRAM
    if _PROGRAM is None:
        _PROGRAM = _build_program()
    return _PROGRAM


def kernel(x, emb, qkv_w, qkv_b, ada_w, ada_b, proj_w, proj_b, _trace=False):
    global LAST_RESULTS
    import ml_dtypes

    nc = _get_program()

    x = np.asarray(x, np.float32)
    emb = np.asarray(emb, np.float32)
    perm = _perm()
    bf = ml_dtypes.bfloat16
    qkv_wT = np.ascontiguousarray(np.asarray(qkv_w, np.float32)[perm, :].T.astype(bf))
    qkv_b_p = np.ascontiguousarray(np.asarray(qkv_b, np.float32)[perm])
    ada_wT = np.ascontiguousarray(np.asarray(ada_w, np.float32).T.astype(bf))
    ada_b = np.ascontiguousarray(np.asarray(ada_b, np.float32))
    proj_wT = np.ascontiguousarray(np.asarray(proj_w, np.float32).T.astype(bf))
    proj_b = np.ascontiguousarray(np.asarray(proj_b, np.float32))

    gind = np.repeat(np.eye(8, dtype=np.float32), 16, axis=0) / 16.0  # [128, 8]
    gindT = np.ascontiguousarray(np.repeat(np.eye(8, dtype=np.float32), 16, axis=0).T)
    ident = np.eye(128, dtype=bf)
    ones = np.ones((128, 64), dtype=np.float32)

    in_maps = []
    for b in range(B):
        in_maps.append(
            {
                "x": np.ascontiguousarray(x[b].reshape(C, T)),
                "emb": np.ascontiguousarray(emb[b]),
                "qkv_wT": qkv_wT,
                "qkv_b": qkv_b_p,
                "ada_wT": ada_wT,
                "ada_b": ada_b,
                "proj_wT": proj_wT,
                "proj_b": proj_b,
                "gind": gind,
                "gindT": gindT,
                "ident": ident,
                "ones": ones,
            }
        )

    res = run_bass_kernel_spmd(nc, in_maps, list(range(8)), trace=_trace)
    LAST_RESULTS = res
    out = np.stack([res.results[b]["out"] for b in range(B)], axis=0)
    return np.ascontiguousarray(out.reshape(B, C, HH, WW).astype(np.float32))



# revision 12
# speedup vs baseline: 1.3084x; 1.3084x over previous
"""Trainium2 Bass kernel for nn_AttentionBlock (adaLN-modulated GroupNorm attention).

Sharding: data-parallel over batch B=8 -> one batch per NeuronCore (8 cores).
Each core runs the full block for its batch:
  groupnorm(32 groups) -> adaLN modulate -> qkv matmul -> 8-head attention
  (softmax over keys) -> proj matmul -> gated residual.

v2 design (restructured from baseline for PE density / HAM warmth):
  - q, k computed as [cout, T] tiles (type-major permuted channel order) so
    head h's q/k live at partition offset 64*(h%2) of tile h//2.
  - v computed TRANSPOSED directly by the qkv matmul (lhsT = xm chunk,
    rhs = v-weight columns) -> vt tiles [128 s, 8*65]: per head 65 cols =
    [ones | v channels], so PV's U output carries the softmax denominator
    in PARTITION ROW 0 (no ones appended via copies, no PE transposes).
  - exp split: head A of each pair on ScalarE (exact exp), head B on DVE
    via the Schraudolph bit trick (bf16 bits ~= int16(A*s + B)); the
    uniform ~0.3% scale error cancels in softmax normalization.
  - normalize: DMA U row 64 (denom) to partition 0 -> DVE fast reciprocal ->
    gpsimd partition_broadcast -> one DVE multiply straight into a_sb (bf16),
    using DVE's ability to write partition-shifted outputs for head B.
  - proj bias folded into the proj matmul (rank-1: lhsT=pb, rhs=ones);
    gated residual fused in one DVE scalar_tensor_tensor per chunk.
  - PE warmup burst of dummy matmuls before qkv to lift the HAM clock gate.

Matmuls run in bf16 (fp32 PSUM accumulation); groupnorm statistics stay fp32.
"""

import numpy as np

import concourse.bass as bass
import concourse.tile as tile
from concourse import bacc, mybir
from concourse.bass_utils import run_bass_kernel_spmd

AF = mybir.ActivationFunctionType
ALU = mybir.AluOpType
f32 = mybir.dt.float32
bf16 = mybir.dt.bfloat16
i16 = mybir.dt.int16

B, C, HH, WW, E = 8, 512, 32, 32, 512
HEADS, G = 8, 32
T = HH * WW          # 1024
CH = C // HEADS      # 64
NC_ = C // 128       # 4 channel chunks
NT = T // 512        # 2 t-chunks of 512
NS = T // 128        # 8 s-chunks of 128
EPS = 1e-5
WARMUP_MM = 16       # PE warmup matmuls before qkv (HAM clock-gate lift)
EXP_SPLIT = True     # head B exp on DVE (Schraudolph); False -> all on ACT
# Schraudolph: bf16 bits of exp(0.125*s) ~= int16(EXP_A*s + EXP_B)
EXP_A = 128.0 * 0.125 / float(np.log(2.0))
EXP_B = 16256.0 - 128.0 * 0.043


def _perm():
    """new[512*ty + 64*h + r] = orig[192*h + 64*ty + r] (head-major -> type-major)."""
    p = np.empty(3 * C, np.int64)
    for h in range(HEADS):
        for ty in range(3):
            p[512 * ty + 64 * h : 512 * ty + 64 * h + 64] = (
                192 * h + 64 * ty + np.arange(64)
            )
    return p


def _build_program():
    nc = bacc.Bacc("TRN2", target_bir_lowering=False, debug=False, num_devices=8)

    # ---- DRAM parameters (per-core shards; weights replicated, bf16) ----
    x_d = nc.declare_dram_parameter("x", [C, T], f32, isOutput=False)
    emb_d = nc.declare_dram_parameter("emb", [E], f32, isOutput=False)
    qw_d = nc.declare_dram_parameter("qkv_wT", [C, 3 * C], bf16, isOutput=False)
    qb_d = nc.declare_dram_parameter("qkv_b", [3 * C], f32, isOutput=False)
    vb_d = nc.declare_dram_parameter("vbrow", [1, C], bf16, isOutput=False)
    aw_d = nc.declare_dram_parameter("ada_wT", [E, 3 * C], bf16, isOutput=False)
    ab_d = nc.declare_dram_parameter("ada_b", [3 * C], f32, isOutput=False)
    pw_d = nc.declare_dram_parameter("proj_wT", [C, C], bf16, isOutput=False)
    pb_d = nc.declare_dram_parameter("pbrow", [1, C], bf16, isOutput=False)
    gind_d = nc.declare_dram_parameter("gind", [128, 8], f32, isOutput=False)
    gindT_d = nc.declare_dram_parameter("gindT", [8, 128], f32, isOutput=False)
    out_d = nc.declare_dram_parameter("out", [C, T], f32, isOutput=True)

    from contextlib import ExitStack

    with tile.TileContext(nc) as tc, ExitStack() as ctx:
        ctx.enter_context(
            nc.allow_low_precision(reason="bf16 matmul inputs; fp32 accumulate")
        )
        P = ctx.enter_context(tc.tile_pool(name="persist", bufs=1))
        # PSUM: "sc" slots [128,1024] f32 = 2 banks x 2 bufs; "u" = 2 banks x 2
        PSM = ctx.enter_context(tc.tile_pool(name="psm", bufs=2, space="PSUM"))
        PSU = ctx.enter_context(tc.tile_pool(name="psu", bufs=2, space="PSUM"))
        EXPP = ctx.enter_context(tc.tile_pool(name="expp", bufs=4))
        EXPI = ctx.enter_context(tc.tile_pool(name="expi", bufs=4))
        ANP = ctx.enter_context(tc.tile_pool(name="anp", bufs=2))

        # ---- persistent SBUF tiles + input DMAs (ordered by need time) ----
        gind_sb = P.tile([128, 8], f32, tag="gind")
        gindT_sb = P.tile([8, 128], f32, tag="gindT")
        emb_sb = P.tile([128, 4], f32, tag="emb")
        qb_sb = P.tile([128, 12], f32, tag="qb")
        ab_sb = P.tile([128, 12], f32, tag="ab")
        vb_sb = P.tile([1, C], bf16, tag="vb")
        pbr_sb = P.tile([1, C], bf16, tag="pbr")

        nc.sync.dma_start(out=gind_sb, in_=gind_d.ap())
        nc.sync.dma_start(out=gindT_sb, in_=gindT_d.ap())
        nc.sync.dma_start(out=emb_sb, in_=emb_d.ap().rearrange("(f p) -> p f", p=128))
        nc.sync.dma_start(out=qb_sb, in_=qb_d.ap().rearrange("(f p) -> p f", p=128))
        nc.sync.dma_start(out=ab_sb, in_=ab_d.ap().rearrange("(f p) -> p f", p=128))
        nc.sync.dma_start(out=vb_sb, in_=vb_d.ap())
        nc.sync.dma_start(out=pbr_sb, in_=pb_d.ap())

        aw = []
        for j in range(4):
            t_ = P.tile([128, 3 * C], bf16, tag=f"aw{j}", name=f"aw{j}")
            nc.sync.dma_start(out=t_, in_=aw_d.ap()[128 * j : 128 * (j + 1), :])
            aw.append(t_)
        xf = []
        for i in range(NC_):
            t_ = P.tile([128, T], f32, tag=f"xf{i}", name=f"xf{i}")
            nc.sync.dma_start(out=t_, in_=x_d.ap()[128 * i : 128 * (i + 1), :])
            xf.append(t_)
        qw = []
        for j in range(4):
            t_ = P.tile([128, 3 * C], bf16, tag=f"qw{j}", name=f"qw{j}")
            nc.sync.dma_start(out=t_, in_=qw_d.ap()[128 * j : 128 * (j + 1), :])
            qw.append(t_)
        pw = []
        for j in range(4):
            t_ = P.tile([128, C], bf16, tag=f"pw{j}", name=f"pw{j}")
            nc.sync.dma_start(out=t_, in_=pw_d.ap()[128 * j : 128 * (j + 1), :])
            pw.append(t_)

        # small constants built on-chip
        ones_row = P.tile([1, 512], bf16, tag="ones_row")
        nc.vector.memset(ones_row, 1.0)
        dummy_sb = P.tile([128, 512], bf16, tag="dummy")
        nc.vector.memset(dummy_sb, 0.0)
        # vt tiles: per s-chunk [128, 8*65]; cols 65h..65h+63 = head h v
        # channels, col 65h+64 = ones -> U row 64 = softmax denominator
        vt = []
        for si in range(NS):
            t_ = P.tile([128, 8 * 65], bf16, tag=f"vt{si}", name=f"vt{si}")
            v3 = t_[:].rearrange("p (h f) -> p h f", f=65)
            nc.vector.memset(v3[:, :, 64:65], 1.0)
            vt.append(t_)

        # ---- phase 1: adaLN modulation + groupnorm stats ----
        sg_sb = P.tile([128, 4], f32, tag="sg")
        silu_sb = P.tile([128, 4], bf16, tag="silu")
        nc.scalar.activation(sg_sb, emb_sb, AF.Sigmoid)
        nc.vector.tensor_mul(silu_sb, emb_sb, sg_sb)
        # preload the exp table set on ACT while idle
        exwarm = P.tile([1, 4], f32, tag="exwarm")
        nc.scalar.activation(exwarm, sg_sb[0:1, 0:4], AF.Exp)
        # mod^T = silu^T @ ada_wT as [1, 1536], then DRAM-bounce to [128, 12]
        mrow = P.tile([1, 3 * C], f32, tag="mrow")
        for oc in range(3):
            mps = PSM.tile([1, 512], f32, tag="sc", name=f"mps{oc}")
            for j in range(4):
                nc.tensor.matmul(
                    mps,
                    silu_sb[:, j : j + 1],
                    aw[j][:, 512 * oc : 512 * (oc + 1)],
                    start=(j == 0),
                    stop=(j == 3),
                )
            nc.vector.tensor_copy(mrow[:, 512 * oc : 512 * (oc + 1)], mps)
        mod_sb = P.tile([128, 12], f32, tag="mod")
        modp_sb = P.tile([128, 12], f32, tag="modp")
        DP = ctx.enter_context(tc.tile_pool(name="dramp", bufs=1, space="DRAM"))
        mod_scr = DP.tile([1, 3 * C], f32, tag="modscr")
        nc.sync.dma_start(out=mod_scr, in_=mrow)
        nc.sync.dma_start(
            out=modp_sb, in_=mod_scr[0, :].rearrange("(f p) -> p f", p=128)
        )
        nc.vector.tensor_add(mod_sb, modp_sb, ab_sb)

        mv = []
        for i in range(NC_):
            st6 = P.tile([128, 2, 6], f32, tag=f"st6{i}")
            xv = xf[i][:].rearrange("p (s f) -> p s f", f=512)
            for si in range(2):
                nc.vector.bn_stats(st6[:, si, :], xv[:, si, :])
            mv_i = P.tile([128, 2], f32, tag=f"mv{i}")
            nc.vector.bn_aggr(mv_i, st6)
            # E2 = var + mu^2 into col 1
            tm = P.tile([128, 1], f32, tag=f"tmu{i}")
            nc.vector.tensor_mul(tm, mv_i[:, 0:1], mv_i[:, 0:1])
            nc.vector.tensor_add(mv_i[:, 1:2], mv_i[:, 1:2], tm)
            mv.append(mv_i)

        stats8_ps = PSM.tile([8, 8], f32, tag="sc", name="stats8")
        for i in range(NC_):
            nc.tensor.matmul(
                stats8_ps[:, 2 * i : 2 * i + 2], gind_sb, mv[i], start=True, stop=True
            )
        s8 = P.tile([8, 8], f32, tag="s8")
        nc.vector.tensor_copy(s8, stats8_ps)
        musq8 = P.tile([8, 4], f32, tag="musq8")
        var8 = P.tile([8, 4], f32, tag="var8")
        sd8 = P.tile([8, 4], f32, tag="sd8")
        rstd8 = P.tile([8, 4], f32, tag="rstd8")
        for i in range(NC_):
            nc.vector.tensor_mul(
                musq8[:, i : i + 1], s8[:, 2 * i : 2 * i + 1], s8[:, 2 * i : 2 * i + 1]
            )
            nc.vector.tensor_sub(
                var8[:, i : i + 1], s8[:, 2 * i + 1 : 2 * i + 2], musq8[:, i : i + 1]
            )
        eps8 = P.tile([8, 1], f32, tag="eps8")
        nc.vector.memset(eps8, EPS)
        nc.scalar.activation(sd8, var8, AF.Sqrt, bias=eps8)
        nc.vector.reciprocal(rstd8, sd8)

        xm = []
        for i in range(NC_):
            statbc = PSM.tile([128, 2], f32, tag="sc", name=f"statbc{i}")
            nc.tensor.matmul(
                statbc[:, 0:1], gindT_sb, s8[:, 2 * i : 2 * i + 1], start=True, stop=True
            )
            nc.tensor.matmul(
                statbc[:, 1:2], gindT_sb, rstd8[:, i : i + 1], start=True, stop=True
            )
            s1p = P.tile([128, 1], f32, tag=f"s1p{i}")
            A_i = P.tile([128, 1], f32, tag=f"A{i}")
            B_i = P.tile([128, 1], f32, tag=f"B{i}")
            tm2 = P.tile([128, 1], f32, tag=f"tm2{i}")
            nc.vector.tensor_scalar_add(s1p, mod_sb[:, 4 + i : 5 + i], 1.0)
            nc.vector.tensor_mul(A_i, statbc[:, 1:2], s1p)
            nc.vector.tensor_mul(tm2, statbc[:, 0:1], A_i)
            nc.vector.tensor_sub(B_i, mod_sb[:, i : i + 1], tm2)
            xm_i = P.tile([128, T], bf16, tag=f"xm{i}", name=f"xm{i}")
            nc.scalar.activation(xm_i, xf[i], AF.Identity, bias=B_i, scale=A_i)
            xm.append(xm_i)

        # ---- PE warmup burst (lift HAM clock gate before qkv) ----
        for w in range(WARMUP_MM):
            wps = PSM.tile([128, T], f32, tag="sc", name=f"warm{w}")
            nc.tensor.matmul(
                wps[:, 0:512], dummy_sb[:, 0:128], dummy_sb, start=True, stop=True
            )

        # ---- phase 2: q,k [cout, T] + vT [s, c] ----
        # qk_sb[0..3] = q chunks, qk_sb[4..7] = k chunks (type-major perm order)
        qk_sb = [
            P.tile([128, T], bf16, tag=f"qk{m}", name=f"qk{m}") for m in range(8)
        ]
        a_sb = [
            P.tile([128, T], bf16, tag=f"asb{j}", name=f"asb{j}") for j in range(NC_)
        ]
        for blk in range(4):
            for m in (blk, 4 + blk):  # q chunk then k chunk
                ps = PSM.tile([128, T], f32, tag="sc", name=f"qkps{m}")
                for j in range(4):
                    for t in range(NT):
                        nc.tensor.matmul(
                            ps[:, 512 * t : 512 * (t + 1)],
                            qw[j][:, 128 * m : 128 * (m + 1)],
                            xm[j][:, 512 * t : 512 * (t + 1)],
                            start=(j == 0),
                            stop=(j == 3),
                        )
                nc.vector.tensor_scalar_add(qk_sb[m][:], ps, qb_sb[:, m : m + 1])
            for si in (2 * blk, 2 * blk + 1):  # vT chunks
                vps = PSM.tile([128, T], f32, tag="sc", name=f"vtps{si}")
                for j in range(4):
                    nc.tensor.matmul(
                        vps[:, 0:512],
                        xm[j][:, 128 * si : 128 * (si + 1)],
                        qw[j][:, 1024:1536],
                        start=(j == 0),
                        stop=False,
                    )
                nc.tensor.matmul(
                    vps[:, 0:512],
                    ones_row[0:1, 0:128],
                    vb_sb[0:1, :],
                    start=False,
                    stop=True,
                )
                nc.vector.tensor_copy(
                    vt[si][:].rearrange("p (h f) -> p h f", f=65)[:, :, 0:64],
                    vps[:, 0:512].rearrange("p (h f) -> p h f", f=64),
                )

        # ---- phase 3+4: attention per head pair ----
        for hp in range(4):
            heads = (2 * hp, 2 * hp + 1)
            U = {}
            for h in heads:
                U[h] = PSU.tile([65, T], f32, tag="u", name=f"u{h}")
            ex_tiles = {}

            def emit_pv(si):
                for h in heads:
                    ex = ex_tiles.pop((h, si))
                    for t in range(NT):
                        nc.tensor.matmul(
                            U[h][:, 512 * t : 512 * (t + 1)],
                            vt[si][:, 65 * h : 65 * h + 65],
                            ex[:, 512 * t : 512 * (t + 1)],
                            start=(si == 0),
                            stop=(si == NS - 1),
                        )

            for si in range(NS):
                sc_ps = {}
                for h in heads:
                    off = 64 * (h % 2)
                    q_ap = qk_sb[hp][off : off + 64, :]
                    k_ap = qk_sb[4 + hp][off : off + 64, :]
                    sc = PSM.tile([128, T], f32, tag="sc", name=f"sc{hp}_{si}_{h}")
                    for t in range(NT):
                        nc.tensor.matmul(
                            sc[:, 512 * t : 512 * (t + 1)],
                            k_ap[:, 128 * si : 128 * (si + 1)],
                            q_ap[:, 512 * t : 512 * (t + 1)],
                            start=True,
                            stop=True,
                            tile_position=(off, 0),
                        )
                    sc_ps[h] = sc
                # exp: head A on ACT; head B on DVE (Schraudolph) except last si
                ha, hb = heads
                ex = EXPP.tile([128, T], bf16, tag="ex")
                nc.scalar.activation(ex, sc_ps[ha], AF.Exp, scale=0.125)
                ex_tiles[(ha, si)] = ex
                if EXP_SPLIT and si < NS - 1:
                    exb = EXPI.tile([128, T], i16, tag="exi")
                    nc.vector.tensor_scalar(
                        exb[:], sc_ps[hb], EXP_A, EXP_B, ALU.mult, ALU.add
                    )
                    ex_tiles[(hb, si)] = exb[:].bitcast(bf16)
                else:
                    exb = EXPP.tile([128, T], bf16, tag="ex")
                    nc.scalar.activation(exb, sc_ps[hb], AF.Exp, scale=0.125)
                    ex_tiles[(hb, si)] = exb
                if si >= 1:
                    emit_pv(si - 1)
            emit_pv(NS - 1)

            # normalize: a = U[0:64] * (1/denom), denom = U row 64
            # (DMA moves the denom row to partition 0: custom DVE/gpsimd ops
            # ignore AP partition offsets on their inputs)
            for h in heads:
                off = 64 * (h % 2)
                rca = ANP.tile([65, T], f32, tag="rca")
                nc.vector.tensor_copy(rca[64:65, :], U[h][64:65, :])
                rc0 = ANP.tile([1, T], f32, tag="rc0")
                nc.sync.dma_start(out=rc0, in_=rca[64:65, :])
                nc.vector.reciprocal_approx_fast(out=rc0[:], in_=rc0[:])
                rbs = ANP.tile([64, T], f32, tag="rbs")
                nc.gpsimd.partition_broadcast(rbs[:], rc0[:])
                nc.vector.tensor_mul(
                    a_sb[hp][off : off + 64, :], U[h][0:64, :], rbs[:]
                )

        # ---- phase 5: proj (+bias via rank-1) + fused gated residual ----
        for m in range(NC_):
            ps = PSM.tile([128, T], f32, tag="sc", name=f"projps{m}")
            for j in range(4):
                for t in range(NT):
                    nc.tensor.matmul(
                        ps[:, 512 * t : 512 * (t + 1)],
                        pw[j][:, 128 * m : 128 * (m + 1)],
                        a_sb[j][:, 512 * t : 512 * (t + 1)],
                        start=(j == 0),
                        stop=False,
                    )
            for t in range(NT):
                nc.tensor.matmul(
                    ps[:, 512 * t : 512 * (t + 1)],
                    pbr_sb[0:1, 128 * m : 128 * (m + 1)],
                    ones_row[0:1, :],
                    start=False,
                    stop=True,
                )
            # xf = xf + gate * (proj + pb)   (one DVE op, in place)
            nc.vector.scalar_tensor_tensor(
                xf[m][:], ps, mod_sb[:, 8 + m : 9 + m], xf[m][:],
                ALU.mult, ALU.add,
            )
            nc.sync.dma_start(out=out_d.ap()[128 * m : 128 * (m + 1), :], in_=xf[m])

    nc.compile()
    return nc


_PROGRAM = None
LAST_RESULTS = None


def _get_program():
    global _PROGRAM
    if _PROGRAM is None:
        _PROGRAM = _build_program()
    return _PROGRAM


def kernel(x, emb, qkv_w, qkv_b, ada_w, ada_b, proj_w, proj_b, _trace=False):
    global LAST_RESULTS
    import ml_dtypes

    nc = _get_program()

    x = np.asarray(x, np.float32)
    emb = np.asarray(emb, np.float32)
    perm = _perm()
    bf = ml_dtypes.bfloat16
    qkv_wT = np.ascontiguousarray(np.asarray(qkv_w, np.float32)[perm, :].T.astype(bf))
    qkv_b_p = np.ascontiguousarray(np.asarray(qkv_b, np.float32)[perm])
    vbrow = np.ascontiguousarray(qkv_b_p[1024:].astype(bf).reshape(1, C))
    ada_wT = np.ascontiguousarray(np.asarray(ada_w, np.float32).T.astype(bf))
    ada_b = np.ascontiguousarray(np.asarray(ada_b, np.float32))
    proj_wT = np.ascontiguousarray(np.asarray(proj_w, np.float32).T.astype(bf))
    pbrow = np.ascontiguousarray(np.asarray(proj_b, np.float32).astype(bf).reshape(1, C))

    gind = np.repeat(np.eye(8, dtype=np.float32), 16, axis=0) / 16.0  # [128, 8]
    gindT = np.ascontiguousarray(np.repeat(np.eye(8, dtype=np.float32), 16, axis=0).T)

    in_maps = []
    for b in range(B):
        in_maps.append(
            {
                "x": np.ascontiguousarray(x[b].reshape(C, T)),
                "emb": np.ascontiguousarray(emb[b]),
                "qkv_wT": qkv_wT,
                "qkv_b": qkv_b_p,
                "vbrow": vbrow,
                "ada_wT": ada_wT,
                "ada_b": ada_b,
                "proj_wT": proj_wT,
                "pbrow": pbrow,
                "gind": gind,
                "gindT": gindT,
            }
        )

    res = run_bass_kernel_spmd(nc, in_maps, list(range(8)), trace=_trace)
    LAST_RESULTS = res
    out = np.stack([res.results[b]["out"] for b in range(B)], axis=0)
    return np.ascontiguousarray(out.reshape(B, C, HH, WW).astype(np.float32))


# revision 21
# speedup vs baseline: 1.3725x; 1.0489x over previous
"""Trainium2 Bass kernel for nn_AttentionBlock (adaLN-modulated GroupNorm attention).

Sharding: data-parallel over batch B=8 -> one batch per NeuronCore (8 cores).
Each core runs the full block for its batch:
  groupnorm(32 groups) -> adaLN modulate -> qkv matmul -> 8-head attention
  (softmax over keys) -> proj matmul -> gated residual.

v2 design (restructured from baseline for PE density / HAM warmth):
  - q, k computed as [cout, T] tiles (type-major permuted channel order) so
    head h's q/k live at partition offset 64*(h%2) of tile h//2.
  - v computed TRANSPOSED directly by the qkv matmul (lhsT = xm chunk,
    rhs = v-weight columns) -> vt tiles [128 s, 8*65]: per head 65 cols =
    [ones | v channels], so PV's U output carries the softmax denominator
    in PARTITION ROW 0 (no ones appended via copies, no PE transposes).
  - exp split: head A of each pair on ScalarE (exact exp), head B on DVE
    via the Schraudolph bit trick (bf16 bits ~= int16(A*s + B)); the
    uniform ~0.3% scale error cancels in softmax normalization.
  - normalize: ACT copies the denom row (bf16) -> PE rank-1 broadcast matmul
    (e64) -> DVE fast reciprocal of the broadcast -> one DVE multiply straight
    into a_sb (bf16), using DVE's partition-shifted output for head B. The
    whole tail is deferred into the next pair's si==1 slot.
  - proj bias folded into the proj matmul (rank-1: lhsT=pb, rhs=ones);
    gated residual fused in one DVE scalar_tensor_tensor per chunk.
  - PE warmup burst of dummy matmuls before qkv to lift the HAM clock gate.

Matmuls run in bf16 (fp32 PSUM accumulation); groupnorm statistics stay fp32.
"""

import numpy as np

import concourse.bass as bass
import concourse.tile as tile
from concourse import bacc, mybir
from concourse.bass_utils import run_bass_kernel_spmd

AF = mybir.ActivationFunctionType
ALU = mybir.AluOpType
f32 = mybir.dt.float32
bf16 = mybir.dt.bfloat16
i16 = mybir.dt.int16

B, C, HH, WW, E = 8, 512, 32, 32, 512
HEADS, G = 8, 32
T = HH * WW          # 1024
CH = C // HEADS      # 64
NC_ = C // 128       # 4 channel chunks
NT = T // 512        # 2 t-chunks of 512
NS = T // 128        # 8 s-chunks of 128
EPS = 1e-5
WARMUP_PRE = 12      # PE warmup matmuls at t=0 (HAM clock-gate lift)
WARMUP_A = 24        # warmup covering the x-DMA/stats window
WARMUP_B = 14        # warmup bridging xm latency into qkv
EXP_SPLIT = True     # head B exp on DVE (Schraudolph); False -> all on ACT
# Schraudolph: bf16 bits of exp(0.125*s) ~= int16(EXP_A*s + EXP_B)
EXP_A = 128.0 * 0.125 / float(np.log(2.0))
EXP_B = 16256.0 - 128.0 * 0.043


def _perm():
    """new[512*ty + 64*h + r] = orig[192*h + 64*ty + r] (head-major -> type-major)."""
    p = np.empty(3 * C, np.int64)
    for h in range(HEADS):
        for ty in range(3):
            p[512 * ty + 64 * h : 512 * ty + 64 * h + 64] = (
                192 * h + 64 * ty + np.arange(64)
            )
    return p


def _build_program():
    nc = bacc.Bacc("TRN2", target_bir_lowering=False, debug=False, num_devices=8)

    # ---- DRAM parameters (per-core shards; weights replicated, bf16) ----
    x_d = nc.declare_dram_parameter("x", [C, T], f32, isOutput=False)
    emb_d = nc.declare_dram_parameter("emb", [E], f32, isOutput=False)
    qw_d = nc.declare_dram_parameter("qkv_wT", [C, 3 * C], bf16, isOutput=False)
    qb_d = nc.declare_dram_parameter("qkv_b", [3 * C], f32, isOutput=False)
    vb_d = nc.declare_dram_parameter("vbrow", [1, C], bf16, isOutput=False)
    aw_d = nc.declare_dram_parameter("ada_wT", [E, 3 * C], bf16, isOutput=False)
    ab_d = nc.declare_dram_parameter("ada_b", [3 * C], f32, isOutput=False)
    pw_d = nc.declare_dram_parameter("proj_wT", [C, C], bf16, isOutput=False)
    pb_d = nc.declare_dram_parameter("pbrow", [1, C], bf16, isOutput=False)
    gind_d = nc.declare_dram_parameter("gind", [128, 8], f32, isOutput=False)
    gindT_d = nc.declare_dram_parameter("gindT", [8, 128], f32, isOutput=False)
    out_d = nc.declare_dram_parameter("out", [C, T], f32, isOutput=True)

    from contextlib import ExitStack

    with tile.TileContext(nc) as tc, ExitStack() as ctx:
        ctx.enter_context(
            nc.allow_low_precision(reason="bf16 matmul inputs; fp32 accumulate")
        )
        P = ctx.enter_context(tc.tile_pool(name="persist", bufs=1))
        # PSUM: "sc" slots [128,1024] f32 = 2 banks x 2 bufs; "u" = 2 banks x 2
        PSM = ctx.enter_context(tc.tile_pool(name="psm", bufs=2, space="PSUM"))
        PSU = ctx.enter_context(tc.tile_pool(name="psu", bufs=2, space="PSUM"))
        EXPP = ctx.enter_context(tc.tile_pool(name="expp", bufs=4))
        EXPI = ctx.enter_context(tc.tile_pool(name="expi", bufs=4))
        ANP = ctx.enter_context(tc.tile_pool(name="anp", bufs=2))

        # ---- persistent SBUF tiles + input DMAs (ordered by need time) ----
        gind_sb = P.tile([128, 8], f32, tag="gind")
        gindT_sb = P.tile([8, 128], f32, tag="gindT")
        emb_sb = P.tile([128, 4], f32, tag="emb")
        qb_sb = P.tile([128, 12], f32, tag="qb")
        ab_sb = P.tile([128, 12], f32, tag="ab")
        vb_sb = P.tile([1, C], bf16, tag="vb")
        pbr_sb = P.tile([1, C], bf16, tag="pbr")

        nc.sync.dma_start(out=gind_sb, in_=gind_d.ap())
        nc.sync.dma_start(out=gindT_sb, in_=gindT_d.ap())
        nc.sync.dma_start(out=emb_sb, in_=emb_d.ap().rearrange("(f p) -> p f", p=128))
        nc.sync.dma_start(out=qb_sb, in_=qb_d.ap().rearrange("(f p) -> p f", p=128))
        nc.sync.dma_start(out=ab_sb, in_=ab_d.ap().rearrange("(f p) -> p f", p=128))
        nc.sync.dma_start(out=vb_sb, in_=vb_d.ap())
        nc.sync.dma_start(out=pbr_sb, in_=pb_d.ap())

        xf = []
        for i in range(NC_):
            t_ = P.tile([128, T], f32, tag=f"xf{i}", name=f"xf{i}")
            nc.sync.dma_start(out=t_, in_=x_d.ap()[128 * i : 128 * (i + 1), :])
            xf.append(t_)
        aw = []
        for j in range(4):
            t_ = P.tile([128, 3 * C], bf16, tag=f"aw{j}", name=f"aw{j}")
            nc.sync.dma_start(out=t_, in_=aw_d.ap()[128 * j : 128 * (j + 1), :])
            aw.append(t_)
        qw = []
        for j in range(4):
            t_ = P.tile([128, 3 * C], bf16, tag=f"qw{j}", name=f"qw{j}")
            nc.sync.dma_start(out=t_, in_=qw_d.ap()[128 * j : 128 * (j + 1), :])
            qw.append(t_)
        pw = []
        for j in range(4):
            t_ = P.tile([128, C], bf16, tag=f"pw{j}", name=f"pw{j}")
            nc.sync.dma_start(out=t_, in_=pw_d.ap()[128 * j : 128 * (j + 1), :])
            pw.append(t_)

        # small constants built on-chip
        ones_row = P.tile([1, 512], bf16, tag="ones_row")
        nc.vector.memset(ones_row, 1.0)
        dummy_sb = P.tile([128, 512], bf16, tag="dummy")
        nc.vector.memset(dummy_sb, 0.0)
        # e64: [65, 64] with row 64 = ones -> PE broadcast of denominator row
        e64 = P.tile([65, 64], bf16, tag="e64")
        nc.vector.memset(e64, 0.0)
        nc.vector.memset(e64[64:65, :], 1.0)
        # rca: per-head-slot staging for the denominator row (row 64); rows
        # 0:64 stay zero so the e64 broadcast matmul sees no garbage
        rca = {}
        for off in (0, 64):
            t_ = P.tile([65, T], bf16, tag=f"rca{off}", name=f"rca{off}")
            nc.vector.memset(t_, 0.0)
            rca[off] = t_

        def warmup(n, tagp):
            for w in range(n):
                wps = PSM.tile([128, T], f32, tag="sc", name=f"{tagp}{w}")
                nc.tensor.matmul(
                    wps[:, 0:512], dummy_sb[:, 0:128], dummy_sb, start=True, stop=True
                )
        # vt tiles: per s-chunk [128, 8*65]; cols 65h..65h+63 = head h v
        # channels, col 65h+64 = ones -> U row 64 = softmax denominator
        vt = []
        for si in range(NS):
            t_ = P.tile([128, 8 * 65], bf16, tag=f"vt{si}", name=f"vt{si}")
            v3 = t_[:].rearrange("p (h f) -> p h f", f=65)
            nc.vector.memset(v3[:, :, 64:65], 1.0)
            vt.append(t_)

        # ---- phase 1: adaLN modulation + groupnorm stats ----
        warmup(WARMUP_PRE, "wp")  # keep PE busy from t=0 (HAM clock gate)
        sg_sb = P.tile([128, 4], f32, tag="sg")
        silu_sb = P.tile([128, 4], bf16, tag="silu")
        nc.scalar.activation(sg_sb, emb_sb, AF.Sigmoid)
        nc.vector.tensor_mul(silu_sb, emb_sb, sg_sb)
        # preload the exp table set on ACT while idle
        exwarm = P.tile([1, 4], f32, tag="exwarm")
        nc.scalar.activation(exwarm, sg_sb[0:1, 0:4], AF.Exp)
        # mod^T = silu^T @ ada_wT as [1, 1536], then DRAM-bounce to [128, 12]
        mrow = P.tile([1, 3 * C], f32, tag="mrow")
        for oc in range(3):
            mps = PSM.tile([1, 512], f32, tag="sc", name=f"mps{oc}")
            for j in range(4):
                nc.tensor.matmul(
                    mps,
                    silu_sb[:, j : j + 1],
                    aw[j][:, 512 * oc : 512 * (oc + 1)],
                    start=(j == 0),
                    stop=(j == 3),
                )
            nc.vector.tensor_copy(mrow[:, 512 * oc : 512 * (oc + 1)], mps)
        mod_sb = P.tile([128, 12], f32, tag="mod")
        modp_sb = P.tile([128, 12], f32, tag="modp")
        DP = ctx.enter_context(tc.tile_pool(name="dramp", bufs=1, space="DRAM"))
        mod_scr = DP.tile([1, 3 * C], f32, tag="modscr")
        nc.sync.dma_start(out=mod_scr, in_=mrow)
        nc.sync.dma_start(
            out=modp_sb, in_=mod_scr[0, :].rearrange("(f p) -> p f", p=128)
        )
        nc.vector.tensor_add(mod_sb, modp_sb, ab_sb)

        warmup(WARMUP_A, "wa")  # cover the x-DMA + stats window
        mv = []
        for i in range(NC_):
            st6 = P.tile([128, 2, 6], f32, tag=f"st6{i}")
            xv = xf[i][:].rearrange("p (s f) -> p s f", f=512)
            for si in range(2):
                nc.vector.bn_stats(st6[:, si, :], xv[:, si, :])
            mv_i = P.tile([128, 2], f32, tag=f"mv{i}")
            nc.vector.bn_aggr(mv_i, st6)
            # E2 = var + mu^2 into col 1
            tm = P.tile([128, 1], f32, tag=f"tmu{i}")
            nc.vector.tensor_mul(tm, mv_i[:, 0:1], mv_i[:, 0:1])
            nc.vector.tensor_add(mv_i[:, 1:2], mv_i[:, 1:2], tm)
            mv.append(mv_i)

        stats8_ps = PSM.tile([8, 8], f32, tag="sc", name="stats8")
        for i in range(NC_):
            nc.tensor.matmul(
                stats8_ps[:, 2 * i : 2 * i + 2], gind_sb, mv[i], start=True, stop=True
            )
        s8 = P.tile([8, 8], f32, tag="s8")
        nc.vector.tensor_copy(s8, stats8_ps)
        musq8 = P.tile([8, 4], f32, tag="musq8")
        var8 = P.tile([8, 4], f32, tag="var8")
        sd8 = P.tile([8, 4], f32, tag="sd8")
        rstd8 = P.tile([8, 4], f32, tag="rstd8")
        for i in range(NC_):
            nc.vector.tensor_mul(
                musq8[:, i : i + 1], s8[:, 2 * i : 2 * i + 1], s8[:, 2 * i : 2 * i + 1]
            )
            nc.vector.tensor_sub(
                var8[:, i : i + 1], s8[:, 2 * i + 1 : 2 * i + 2], musq8[:, i : i + 1]
            )
        eps8 = P.tile([8, 1], f32, tag="eps8")
        nc.vector.memset(eps8, EPS)
        nc.scalar.activation(sd8, var8, AF.Sqrt, bias=eps8)
        nc.vector.reciprocal(rstd8, sd8)

        xm = []
        for i in range(NC_):
            statbc = PSM.tile([128, 2], f32, tag="sc", name=f"statbc{i}")
            nc.tensor.matmul(
                statbc[:, 0:1], gindT_sb, s8[:, 2 * i : 2 * i + 1], start=True, stop=True
            )
            nc.tensor.matmul(
                statbc[:, 1:2], gindT_sb, rstd8[:, i : i + 1], start=True, stop=True
            )
            s1p = P.tile([128, 1], f32, tag=f"s1p{i}")
            A_i = P.tile([128, 1], f32, tag=f"A{i}")
            B_i = P.tile([128, 1], f32, tag=f"B{i}")
            tm2 = P.tile([128, 1], f32, tag=f"tm2{i}")
            nc.vector.tensor_scalar_add(s1p, mod_sb[:, 4 + i : 5 + i], 1.0)
            nc.vector.tensor_mul(A_i, statbc[:, 1:2], s1p)
            nc.vector.tensor_mul(tm2, statbc[:, 0:1], A_i)
            nc.vector.tensor_sub(B_i, mod_sb[:, i : i + 1], tm2)
            xm_i = P.tile([128, T], bf16, tag=f"xm{i}", name=f"xm{i}")
            nc.scalar.activation(xm_i, xf[i], AF.Identity, bias=B_i, scale=A_i)
            xm.append(xm_i)

        warmup(WARMUP_B, "wb")  # bridge the xm-activation latency into qkv

        # ---- phase 2: q,k [cout, T] + vT [s, c] ----
        # qk_sb[0..3] = q chunks, qk_sb[4..7] = k chunks (type-major perm order)
        qk_sb = [
            P.tile([128, T], bf16, tag=f"qk{m}", name=f"qk{m}") for m in range(8)
        ]
        a_sb = [
            P.tile([128, T], bf16, tag=f"asb{j}", name=f"asb{j}") for j in range(NC_)
        ]
        for blk in range(4):
            for m in (blk, 4 + blk):  # q chunk then k chunk
                ps = PSM.tile([128, T], f32, tag="sc", name=f"qkps{m}")
                for j in range(4):
                    for t in range(NT):
                        nc.tensor.matmul(
                            ps[:, 512 * t : 512 * (t + 1)],
                            qw[j][:, 128 * m : 128 * (m + 1)],
                            xm[j][:, 512 * t : 512 * (t + 1)],
                            start=(j == 0),
                            stop=(j == 3),
                        )
                # eviction on ACT (idle during phase 2): out = ps + qb
                nc.scalar.activation(
                    qk_sb[m][:], ps, AF.Identity, bias=qb_sb[:, m : m + 1]
                )
            for si in (2 * blk, 2 * blk + 1):  # vT chunks
                vps = PSM.tile([128, T], f32, tag="sc", name=f"vtps{si}")
                for j in range(4):
                    nc.tensor.matmul(
                        vps[:, 0:512],
                        xm[j][:, 128 * si : 128 * (si + 1)],
                        qw[j][:, 1024:1536],
                        start=(j == 0),
                        stop=False,
                    )
                nc.tensor.matmul(
                    vps[:, 0:512],
                    ones_row[0:1, 0:128],
                    vb_sb[0:1, :],
                    start=False,
                    stop=True,
                )
                nc.vector.tensor_copy(
                    vt[si][:].rearrange("p (h f) -> p h f", f=65)[:, :, 0:64],
                    vps[:, 0:512].rearrange("p (h f) -> p h f", f=64),
                )

        # ---- phase 3+4: attention per head pair ----
        # Normalize (a = U[0:64] / U[64]) is deferred into the NEXT pair's
        # si==1 slot so the chain (ACT copy of the denom row -> PE e64
        # broadcast matmul -> DVE reciprocal -> DVE multiply into a_sb)
        # overlaps the next pair's scores instead of stalling the PE.
        pending = None  # (hp, U) awaiting normalize tail

        def emit_norm_tail():
            nonlocal pending
            if pending is None:
                return
            php, pU = pending
            pending = None
            for h in (2 * php, 2 * php + 1):
                off = 64 * (h % 2)
                bc = PSM.tile([64, T], f32, tag="sc", name=f"bc{php}_{h}")
                for t in range(NT):
                    nc.tensor.matmul(
                        bc[:, 512 * t : 512 * (t + 1)],
                        e64[:, 0:64],
                        rca[off][0:65, 512 * t : 512 * (t + 1)],
                        start=True,
                        stop=True,
                    )
                rbs = ANP.tile([64, T], f32, tag="rbs")
                nc.vector.reciprocal_approx_fast(out=rbs[:], in_=bc[:])
                nc.vector.tensor_mul(
                    a_sb[php][off : off + 64, :], pU[h][0:64, :], rbs[:]
                )

        for hp in range(4):
            heads = (2 * hp, 2 * hp + 1)
            U = {}
            for h in heads:
                U[h] = PSU.tile([65, T], f32, tag="u", name=f"u{hp}_{h}")
            ex_tiles = {}

            def emit_pv(si):
                for h in heads:
                    ex = ex_tiles.pop((h, si))
                    for t in range(NT):
                        nc.tensor.matmul(
                            U[h][:, 512 * t : 512 * (t + 1)],
                            vt[si][:, 65 * h : 65 * h + 65],
                            ex[:, 512 * t : 512 * (t + 1)],
                            start=(si == 0),
                            stop=(si == NS - 1),
                        )

            for si in range(NS):
                sc_ps = {}
                for h in heads:
                    off = 64 * (h % 2)
                    q_ap = qk_sb[hp][off : off + 64, :]
                    k_ap = qk_sb[4 + hp][off : off + 64, :]
                    sc = PSM.tile([128, T], f32, tag="sc", name=f"sc{hp}_{si}_{h}")
                    for t in range(NT):
                        nc.tensor.matmul(
                            sc[:, 512 * t : 512 * (t + 1)],
                            k_ap[:, 128 * si : 128 * (si + 1)],
                            q_ap[:, 512 * t : 512 * (t + 1)],
                            start=True,
                            stop=True,
                            tile_position=(off, 0),
                        )
                    sc_ps[h] = sc
                # exp: head A on ACT (exact); head B on DVE (Schraudolph)
                ha, hb = heads
                ex = EXPP.tile([128, T], bf16, tag="ex")
                nc.scalar.activation(ex, sc_ps[ha], AF.Exp, scale=0.125)
                ex_tiles[(ha, si)] = ex
                if EXP_SPLIT:
                    exb = EXPI.tile([128, T], i16, tag="exi")
                    nc.vector.tensor_scalar(
                        exb[:], sc_ps[hb], EXP_A, EXP_B, ALU.mult, ALU.add
                    )
                    ex_tiles[(hb, si)] = exb[:].bitcast(bf16)
                else:
                    exb = EXPP.tile([128, T], bf16, tag="ex")
                    nc.scalar.activation(exb, sc_ps[hb], AF.Exp, scale=0.125)
                    ex_tiles[(hb, si)] = exb
                if si == 1:
                    emit_norm_tail()
                if si >= 2:
                    emit_pv(si - 2)
            emit_pv(NS - 2)
            emit_pv(NS - 1)

            # stage the denominator rows to SBUF (ACT; rows 0:64 stay zero)
            for h in heads:
                off = 64 * (h % 2)
                nc.scalar.copy(rca[off][64:65, :], U[h][64:65, :])
            pending = (hp, U)
        emit_norm_tail()

        # ---- phase 5: proj (+bias via rank-1) + fused gated residual ----
        for m in range(NC_):
            ps = PSM.tile([128, T], f32, tag="sc", name=f"projps{m}")
            for j in range(4):
                for t in range(NT):
                    nc.tensor.matmul(
                        ps[:, 512 * t : 512 * (t + 1)],
                        pw[j][:, 128 * m : 128 * (m + 1)],
                        a_sb[j][:, 512 * t : 512 * (t + 1)],
                        start=(j == 0),
                        stop=False,
                    )
            for t in range(NT):
                nc.tensor.matmul(
                    ps[:, 512 * t : 512 * (t + 1)],
                    pbr_sb[0:1, 128 * m : 128 * (m + 1)],
                    ones_row[0:1, :],
                    start=False,
                    stop=True,
                )
            # xf = xf + gate * (proj + pb)   (one DVE op, in place)
            nc.vector.scalar_tensor_tensor(
                xf[m][:], ps, mod_sb[:, 8 + m : 9 + m], xf[m][:],
                ALU.mult, ALU.add,
            )
            nc.sync.dma_start(out=out_d.ap()[128 * m : 128 * (m + 1), :], in_=xf[m])

    nc.compile()
    return nc


_PROGRAM = None
LAST_RESULTS = None


def _get_program():
    global _PROGRAM
    if _PROGRAM is None:
        _PROGRAM = _build_program()
    return _PROGRAM


def kernel(x, emb, qkv_w, qkv_b, ada_w, ada_b, proj_w, proj_b, _trace=False):
    global LAST_RESULTS
    import ml_dtypes

    nc = _get_program()

    x = np.asarray(x, np.float32)
    emb = np.asarray(emb, np.float32)
    perm = _perm()
    bf = ml_dtypes.bfloat16
    qkv_wT = np.ascontiguousarray(np.asarray(qkv_w, np.float32)[perm, :].T.astype(bf))
    qkv_b_p = np.ascontiguousarray(np.asarray(qkv_b, np.float32)[perm])
    vbrow = np.ascontiguousarray(qkv_b_p[1024:].astype(bf).reshape(1, C))
    ada_wT = np.ascontiguousarray(np.asarray(ada_w, np.float32).T.astype(bf))
    ada_b = np.ascontiguousarray(np.asarray(ada_b, np.float32))
    proj_wT = np.ascontiguousarray(np.asarray(proj_w, np.float32).T.astype(bf))
    pbrow = np.ascontiguousarray(np.asarray(proj_b, np.float32).astype(bf).reshape(1, C))

    gind = np.repeat(np.eye(8, dtype=np.float32), 16, axis=0) / 16.0  # [128, 8]
    gindT = np.ascontiguousarray(np.repeat(np.eye(8, dtype=np.float32), 16, axis=0).T)

    in_maps = []
    for b in range(B):
        in_maps.append(
            {
                "x": np.ascontiguousarray(x[b].reshape(C, T)),
                "emb": np.ascontiguousarray(emb[b]),
                "qkv_wT": qkv_wT,
                "qkv_b": qkv_b_p,
                "vbrow": vbrow,
                "ada_wT": ada_wT,
                "ada_b": ada_b,
                "proj_wT": proj_wT,
                "pbrow": pbrow,
                "gind": gind,
                "gindT": gindT,
            }
        )

    res = run_bass_kernel_spmd(nc, in_maps, list(range(8)), trace=_trace)
    LAST_RESULTS = res
    out = np.stack([res.results[b]["out"] for b in range(B)], axis=0)
    return np.ascontiguousarray(out.reshape(B, C, HH, WW).astype(np.float32))
